# revision 12
# baseline (speedup 1.0000x reference)
"""Euclidean distance block (retrieval kNN) on 8 TRN2 NeuronCores.

dist[b, s, p] = sqrt(sum_c (x1[b, c, p] - x2[b, s, c, p])^2)   p = spatial (h*w)
out[b] = dist[b].reshape(S * h * w)

Sharding: data-parallel over batch B=32 -> 4 batches per core, no comms.

Per-core kernel layout: SBUF partitions carry (support_pair, channel) = 2*64 =
128; the free axis carries spatial. A big tile covers 8 supports as
[128, 4, 1764], streamed as four fully-contiguous 902 KB pair-DMAs (f32 HBM
-> bf16 SBUF cast on the SWDGE ring; per-pair DMAs give 4x finer completion
sems so compute starts on the first pair). The SWDGE load stream saturates
the HBM/NC limit (~389 GB/s measured) from ~7us to the end, so everything
else is scheduled to stay off that ring. Compute chain per tile:
  DVE subtract in bf16 (2x mode), in place
  Square -> bf16: 3 slices on ACT, 1 on DVE (engine cadence balance)
  PE: 4 per-quarter matmuls per support pair against a [128, 25] one-hot
    pair mask into a [25, 4, 512] PSUM tile (one bank per quarter; a single
    multi-bank matmul is invalid ISA), accumulating per-support sums over C
  ACT Sqrt PSUM -> SBUF f32 per quarter as soon as that quarter's sum
    completes (spreads sqrt into the matmul stream), then one contiguous
    per-batch store on the Sync HWDGE ring (7 KB descriptors; finer strided
    stores measurably steal HBM bandwidth from the load stream).

The last pair of each batch's last tile is q-sliced (sub/square/matmul per
441-wide quarter, sqrt+store interleaved) so the kernel tail after the last
HBM byte is one quarter-chain, and for the last batch that pair's load is
split into four 225 KB chunk-DMAs so the chain overlaps the final DMAs.
"""

import numpy as np

B, S, C, H, W = 32, 25, 64, 42, 42
HW = H * W            # 1764
NCORES = 8
BL = B // NCORES      # 4 batches per core
NSO = 4               # support pairs per big tile (8 supports)
NBIG = 3              # big tiles per batch (24 supports), then 1 leftover
NQ = 4                # spatial quarters
QW = HW // NQ         # 441
NPAIR = 13            # 12 support pairs + 1 leftover single
PSW = 512             # PSUM bank stride in f32 elements

_cache = {}


def _build_nc():
    import concourse.bacc as bacc
    import concourse.mybir as mybir
    from concourse.tile import TileContext
    from concourse.bass import MemorySpace

    f32 = mybir.dt.float32
    bf16 = mybir.dt.bfloat16
    Square = mybir.ActivationFunctionType.Square
    Sqrt = mybir.ActivationFunctionType.Sqrt
    sub = mybir.AluOpType.subtract
    mult = mybir.AluOpType.mult

    # Square and Sqrt both live in the "sqrt_and_others" act-function set,
    # but the table-load chooser picks the first set containing each one,
    # alternating two ~2.7us table reloads per batch. Strip the two
    # functions from every other set (contents only — set ids are
    # positional) so one resident table serves the whole kernel.
    _orig_tables = bacc.get_activation_tables

    def _pinned_tables(arch):
        t = _orig_tables(arch)
        for name, fns in t.items():
            if name != "sqrt_and_others":
                fns.discard(Square)
                fns.discard(Sqrt)
        return t

    bacc.get_activation_tables = _pinned_tables
    nc = bacc.Bacc()
    x1 = nc.declare_dram_parameter("x1", [BL, C, HW], f32, isOutput=False)
    x2 = nc.declare_dram_parameter("x2", [BL, S, C, HW], f32, isOutput=False)
    mk = nc.declare_dram_parameter("mask", [NPAIR, 128, S], bf16, isOutput=False)
    out = nc.declare_dram_parameter("out", [BL, S * HW], f32, isOutput=True)

    with TileContext(nc) as tc:
        with (
            tc.tile_pool(name="x2p", bufs=6) as x2p,
            tc.tile_pool(name="sqp", bufs=3) as sqp,
            tc.tile_pool(name="x1p", bufs=1) as x1p,
            tc.tile_pool(name="outp", bufs=2) as outp,
            tc.tile_pool(name="cst", bufs=1) as cst,
            tc.tile_pool(name="ps", bufs=2, space=MemorySpace.PSUM) as psp,
        ):
            mt = cst.tile([128, NPAIR, S], bf16)
            nc.sync.dma_start(mt[:], mk.rearrange("g k m -> k g m"))

            # all of x1 once: [c, b, p] on partitions 0..63, then duplicate
            # onto 64..127 via SBUF->SBUF (no extra HBM traffic)
            x1all = x1p.tile([128, BL, HW], bf16)
            nc.gpsimd.dma_start(x1all[0:64, :, :], x1.rearrange("b c p -> c b p"))
            nc.sync.dma_start(x1all[64:128, :, :], x1all[0:64, :, :])

            for b in range(BL):
                last_b = b == BL - 1

                # leftover support 24: DMA early so it streams with big tiles
                x2l = x2p.tile([64, HW], bf16, tag="x2l")
                nc.gpsimd.dma_start(x2l[:], x2[b, S - 1])

                # one PSUM tile spanning 4 banks; quarter q lives at
                # [:, q, 0:QW] so one compound matmul covers all quarters
                ps = psp.tile([S, NQ, PSW], f32, name=f"ps{b}", tag="ps")

                # leftover compute first keeps the end-of-batch tail short
                nc.vector.tensor_tensor(x2l[:], x2l[:], x1all[0:64, b, :], sub)
                sql = sqp.tile([64, HW], bf16, name="sql", tag="sql")
                nc.scalar.activation(sql[:], x2l[:], Square)
                for q in range(NQ):
                    nc.tensor.matmul(
                        ps[:, q, 0:QW],
                        mt[0:64, NPAIR - 1, :],
                        sql[:, q * QW : (q + 1) * QW],
                        start=True,
                        stop=False,
                    )

                ot = outp.tile([S, HW], f32, name="ot", tag="ot")
                for i in range(NBIG):
                    last_tile = i == NBIG - 1
                    x2t = x2p.tile([128, NSO, HW], bf16, tag="x2t")
                    src = x2[b, 8 * i : 8 * i + 8].rearrange(
                        "(so si) c p -> (si c) so p", si=2
                    )
                    # per-pair DMAs: same streaming rate, but 4x finer
                    # completion sems -> subs start on the first 902KB.
                    # The very last pair of the kernel is chunked 4x finer
                    # still, so its compute overlaps the final DMAs.
                    for so in range(NSO):
                        if last_b and last_tile and so == NSO - 1:
                            for q in range(NQ):
                                qs = slice(q * QW, (q + 1) * QW)
                                nc.gpsimd.dma_start(
                                    x2t[:, so, qs], src[:, so, qs]
                                )
                        else:
                            nc.gpsimd.dma_start(x2t[:, so, :], src[:, so, :])
                    sq = sqp.tile([128, NSO, HW], bf16, tag="sq")
                    for so in range(NSO):
                        j = NSO * i + so
                        xt = x2t[:, so, :]
                        if not (last_tile and so == NSO - 1):
                            # in-place: x2t slice becomes diff
                            nc.vector.tensor_tensor(xt, xt, x1all[:, b, :], sub)
                            # squares split 3/1 across ACT and DVE to balance
                            # the per-tile engine cadence
                            if so < 3:
                                nc.scalar.activation(sq[:, so, :], xt, Square)
                            else:
                                nc.vector.tensor_tensor(sq[:, so, :], xt, xt, mult)
                            for q in range(NQ):
                                nc.tensor.matmul(
                                    ps[:, q, 0:QW],
                                    mt[:, j, :],
                                    sq[:, so, q * QW : (q + 1) * QW],
                                    start=False,
                                    stop=False,
                                )
                        else:
                            # final pair: per-quarter chain, sqrt (+store on
                            # the last batch) fired as each quarter's sum
                            # completes
                            for q in range(NQ):
                                qs = slice(q * QW, (q + 1) * QW)
                                nc.vector.tensor_tensor(
                                    xt[:, qs], xt[:, qs], x1all[:, b, qs], sub
                                )
                                if q % 2 == 0:
                                    nc.scalar.activation(
                                        sq[:, so, qs], xt[:, qs], Square
                                    )
                                else:
                                    nc.vector.tensor_tensor(
                                        sq[:, so, qs], xt[:, qs], xt[:, qs], mult
                                    )
                                nc.tensor.matmul(
                                    ps[:, q, 0:QW],
                                    mt[:, j, :],
                                    sq[:, so, qs],
                                    start=False,
                                    stop=True,
                                )
                                nc.scalar.activation(ot[:, qs], ps[:, q, 0:QW], Sqrt)
                                if last_b:
                                    nc.sync.dma_start(
                                        out[b].rearrange("(s p) -> s p", s=S)[:, qs],
                                        ot[:, qs],
                                    )

                if not last_b:
                    # one contiguous 176KB store on the Sync HWDGE ring:
                    # coarse 7KB descriptors; finer strided stores steal HBM
                    # bandwidth from the saturated SWDGE load stream
                    nc.sync.dma_start(out[b].rearrange("(s p) -> s p", s=S), ot[:])

    try:
        nc.finalize()
    finally:
        bacc.get_activation_tables = _orig_tables
    return nc


def get_nc():
    if "nc" not in _cache:
        _cache["nc"] = _build_nc()
    return _cache["nc"]


def make_mask() -> np.ndarray:
    # mask[j, k, m] = 1 iff partition k of pair-tile j feeds output support m.
    # Pair j < 12 covers supports (2j, 2j+1): k < 64 -> 2j, k >= 64 -> 2j+1.
    # Pair 12 is the leftover single support 24 on partitions 0..63.
    import ml_dtypes

    mask = np.zeros((NPAIR, 128, S), dtype=ml_dtypes.bfloat16)
    for j in range(NPAIR - 1):
        mask[j, 0:64, 2 * j] = 1.0
        mask[j, 64:128, 2 * j + 1] = 1.0
    mask[NPAIR - 1, 0:64, S - 1] = 1.0
    return mask


def make_in_maps(x1: np.ndarray, x2: np.ndarray) -> list[dict]:
    x1 = np.ascontiguousarray(np.asarray(x1, dtype=np.float32)).reshape(B, C, HW)
    x2 = np.ascontiguousarray(np.asarray(x2, dtype=np.float32)).reshape(B, S, C, HW)
    mask = make_mask()
    maps = []
    for i in range(NCORES):
        sl = slice(i * BL, (i + 1) * BL)
        maps.append({"x1": x1[sl], "x2": x2[sl], "mask": mask})
    return maps


def gather_out(results: list[dict]) -> np.ndarray:
    return np.concatenate([np.asarray(r["out"]) for r in results], axis=0).astype(
        np.float32, copy=False
    )


def kernel(x1, x2) -> np.ndarray:
    from concourse.bass_utils import run_bass_kernel_spmd

    nc = get_nc()
    in_maps = make_in_maps(x1, x2)
    res = run_bass_kernel_spmd(nc, in_maps, list(range(NCORES)))
    return gather_out(res.results)


# revision 13
# speedup vs baseline: 1.1096x; 1.1096x over previous
"""Euclidean distance block (retrieval kNN) on 8 TRN2 NeuronCores.

dist[b, s, p] = sqrt(sum_c (x1[b, c, p] - x2[b, s, c, p])^2)   p = spatial (h*w)
out[b] = dist[b].reshape(S * h * w)

Sharding: data-parallel over batch B=32 -> 4 batches per core, no comms.

Per-core kernel layout: SBUF partitions carry (support_pair, channel) = 2*64 =
128; the free axis carries spatial. A big tile covers 8 supports as
[128, 4, 1764], streamed as four fully-contiguous 902 KB pair-DMAs (f32 HBM
-> bf16 SBUF cast on the SWDGE ring; per-pair DMAs give 4x finer completion
sems so compute starts on the first pair). The SWDGE load stream saturates
the HBM/NC limit (~389 GB/s measured) from ~7us to ~128us; compute is
arrival-paced behind it, so the only schedule-sensitive spans are the fixed
preamble and the tail after the last HBM byte. Compute chain per tile:
  DVE subtract in bf16 (2x mode), in place
  Square -> bf16: 3 slices on ACT, 1 on DVE (engine cadence balance)
  PE matmul against [128, 25] one-hot pair masks, accumulating per-support
    sums over C into four [25, 441] PSUM tiles (one bank per spatial quarter;
    a single multi-bank matmul is invalid ISA)
  ACT Sqrt PSUM -> SBUF f32, one contiguous 176 KB store per batch on the
    idle Sync HWDGE ring (coarse 7 KB descriptors; finer strided stores
    measurably steal HBM bandwidth from the load stream, and a store on the
    ACT ring would block ACT for ~0.7us).

Tail: only the kernel's very last pair is q-sliced — its load arrives as four
225 KB chunk-DMAs, and per quarter the chain sub -> square -> matmul -> sqrt
-> store overlaps the remaining chunks. The squares of that pair run on DVE
ONLY: putting a square between two sqrts in ACT program order chains
MM_q -> sqrt_q -> square_{q+1} -> MM_{q+1} serially (~2.3us per quarter,
measured), while DVE-only squares keep ACT's order [sqrt0..sqrt3] and the
quarters pipeline at ~0.8us.
"""

import numpy as np

B, S, C, H, W = 32, 25, 64, 42, 42
HW = H * W            # 1764
NCORES = 8
BL = B // NCORES      # 4 batches per core
NSO = 4               # support pairs per big tile (8 supports)
NBIG = 3              # big tiles per batch (24 supports), then 1 leftover
NQ = 4                # spatial quarters
QW = HW // NQ         # 441
NPAIR = 13            # 12 support pairs + 1 leftover single

_cache = {}


def _build_nc():
    import concourse.bacc as bacc
    import concourse.mybir as mybir
    from concourse.tile import TileContext
    from concourse.bass import MemorySpace

    f32 = mybir.dt.float32
    bf16 = mybir.dt.bfloat16
    Square = mybir.ActivationFunctionType.Square
    Sqrt = mybir.ActivationFunctionType.Sqrt
    sub = mybir.AluOpType.subtract
    mult = mybir.AluOpType.mult

    # Square and Sqrt both live in the "sqrt_and_others" act-function set,
    # but the table-load chooser picks the first set containing each one,
    # alternating two ~2.7us table reloads per batch. Strip the two
    # functions from every other set (contents only — set ids are
    # positional) so one resident table serves the whole kernel.
    _orig_tables = bacc.get_activation_tables

    def _pinned_tables(arch):
        t = _orig_tables(arch)
        for name, fns in t.items():
            if name != "sqrt_and_others":
                fns.discard(Square)
                fns.discard(Sqrt)
        return t

    bacc.get_activation_tables = _pinned_tables
    nc = bacc.Bacc()
    x1 = nc.declare_dram_parameter("x1", [BL, C, HW], f32, isOutput=False)
    x2 = nc.declare_dram_parameter("x2", [BL, S, C, HW], f32, isOutput=False)
    mk = nc.declare_dram_parameter("mask", [NPAIR, 128, S], bf16, isOutput=False)
    out = nc.declare_dram_parameter("out", [BL, S * HW], f32, isOutput=True)

    with TileContext(nc) as tc:
        with (
            tc.tile_pool(name="x2p", bufs=6) as x2p,
            tc.tile_pool(name="sqp", bufs=3) as sqp,
            tc.tile_pool(name="x1p", bufs=1) as x1p,
            tc.tile_pool(name="outp", bufs=2) as outp,
            tc.tile_pool(name="cst", bufs=1) as cst,
            tc.tile_pool(name="ps", bufs=2, space=MemorySpace.PSUM) as psp,
        ):
            mt = cst.tile([128, NPAIR, S], bf16)
            nc.sync.dma_start(mt[:], mk.rearrange("g k m -> k g m"))

            # all of x1 once: [c, b, p] on partitions 0..63, then duplicate
            # onto 64..127 via SBUF->SBUF (no extra HBM traffic)
            x1all = x1p.tile([128, BL, HW], bf16)
            nc.gpsimd.dma_start(x1all[0:64, :, :], x1.rearrange("b c p -> c b p"))
            nc.sync.dma_start(x1all[64:128, :, :], x1all[0:64, :, :])

            for b in range(BL):
                last_b = b == BL - 1

                # leftover support 24: DMA early so it streams with big tiles
                x2l = x2p.tile([64, HW], bf16, tag="x2l")
                nc.gpsimd.dma_start(x2l[:], x2[b, S - 1])

                pst = [
                    psp.tile([S, QW], f32, name=f"ps{q}", tag=f"ps{q}")
                    for q in range(NQ)
                ]

                # leftover compute first keeps the end-of-batch tail short
                nc.vector.tensor_tensor(x2l[:], x2l[:], x1all[0:64, b, :], sub)
                sql = sqp.tile([64, HW], bf16, name="sql", tag="sql")
                nc.scalar.activation(sql[:], x2l[:], Square)
                for q in range(NQ):
                    nc.tensor.matmul(
                        pst[q][:, :],
                        mt[0:64, NPAIR - 1, :],
                        sql[:, q * QW : (q + 1) * QW],
                        start=True,
                        stop=False,
                    )

                ot = outp.tile([S, HW], f32, name="ot", tag="ot")
                for i in range(NBIG):
                    x2t = x2p.tile([128, NSO, HW], bf16, tag="x2t")
                    src = x2[b, 8 * i : 8 * i + 8].rearrange(
                        "(so si) c p -> (si c) so p", si=2
                    )
                    # per-pair DMAs: same streaming rate, but 4x finer
                    # completion sems -> subs start on the first 902KB.
                    # The kernel's very last pair is chunked 4x finer still,
                    # so its per-quarter chain overlaps the final DMAs.
                    for so in range(NSO):
                        if last_b and i == NBIG - 1 and so == NSO - 1:
                            for q in range(NQ):
                                qs = slice(q * QW, (q + 1) * QW)
                                nc.gpsimd.dma_start(
                                    x2t[:, so, qs], src[:, so, qs]
                                )
                        else:
                            nc.gpsimd.dma_start(x2t[:, so, :], src[:, so, :])
                    sq = sqp.tile([128, NSO, HW], bf16, tag="sq")
                    for so in range(NSO):
                        j = NSO * i + so
                        xt = x2t[:, so, :]
                        stop = j == NPAIR - 2
                        if not (last_b and i == NBIG - 1 and so == NSO - 1):
                            # in-place: x2t slice becomes diff
                            nc.vector.tensor_tensor(xt, xt, x1all[:, b, :], sub)
                            # squares split 3/1 across ACT and DVE to balance
                            # the per-tile engine cadence
                            if so < 3:
                                nc.scalar.activation(sq[:, so, :], xt, Square)
                            else:
                                nc.vector.tensor_tensor(sq[:, so, :], xt, xt, mult)
                            for q in range(NQ):
                                nc.tensor.matmul(
                                    pst[q][:, :],
                                    mt[:, j, :],
                                    sq[:, so, q * QW : (q + 1) * QW],
                                    start=False,
                                    stop=stop,
                                )
                        else:
                            # kernel tail: per-quarter chain with DVE-only
                            # squares (see module docstring), sqrt + store
                            # fired as each quarter's sum completes
                            for q in range(NQ):
                                qs = slice(q * QW, (q + 1) * QW)
                                nc.vector.tensor_tensor(
                                    xt[:, qs], xt[:, qs], x1all[:, b, qs], sub
                                )
                                nc.vector.tensor_tensor(
                                    sq[:, so, qs], xt[:, qs], xt[:, qs], mult
                                )
                                nc.tensor.matmul(
                                    pst[q][:, :],
                                    mt[:, j, :],
                                    sq[:, so, qs],
                                    start=False,
                                    stop=True,
                                )
                                nc.scalar.activation(ot[:, qs], pst[q][:], Sqrt)
                                nc.sync.dma_start(
                                    out[b].rearrange("(s p) -> s p", s=S)[:, qs],
                                    ot[:, qs],
                                )

                if not last_b:
                    for q in range(NQ):
                        nc.scalar.activation(
                            ot[:, q * QW : (q + 1) * QW], pst[q][:], Sqrt
                        )
                    # one contiguous 176KB store on the idle Sync HWDGE ring
                    nc.sync.dma_start(out[b].rearrange("(s p) -> s p", s=S), ot[:])

    try:
        nc.finalize()
    finally:
        bacc.get_activation_tables = _orig_tables
    return nc


def get_nc():
    if "nc" not in _cache:
        _cache["nc"] = _build_nc()
    return _cache["nc"]


def make_mask() -> np.ndarray:
    # mask[j, k, m] = 1 iff partition k of pair-tile j feeds output support m.
    # Pair j < 12 covers supports (2j, 2j+1): k < 64 -> 2j, k >= 64 -> 2j+1.
    # Pair 12 is the leftover single support 24 on partitions 0..63.
    import ml_dtypes

    mask = np.zeros((NPAIR, 128, S), dtype=ml_dtypes.bfloat16)
    for j in range(NPAIR - 1):
        mask[j, 0:64, 2 * j] = 1.0
        mask[j, 64:128, 2 * j + 1] = 1.0
    mask[NPAIR - 1, 0:64, S - 1] = 1.0
    return mask


def make_in_maps(x1: np.ndarray, x2: np.ndarray) -> list[dict]:
    x1 = np.ascontiguousarray(np.asarray(x1, dtype=np.float32)).reshape(B, C, HW)
    x2 = np.ascontiguousarray(np.asarray(x2, dtype=np.float32)).reshape(B, S, C, HW)
    mask = make_mask()
    maps = []
    for i in range(NCORES):
        sl = slice(i * BL, (i + 1) * BL)
        maps.append({"x1": x1[sl], "x2": x2[sl], "mask": mask})
    return maps


def gather_out(results: list[dict]) -> np.ndarray:
    return np.concatenate([np.asarray(r["out"]) for r in results], axis=0).astype(
        np.float32, copy=False
    )


def kernel(x1, x2) -> np.ndarray:
    from concourse.bass_utils import run_bass_kernel_spmd

    nc = get_nc()
    in_maps = make_in_maps(x1, x2)
    res = run_bass_kernel_spmd(nc, in_maps, list(range(NCORES)))
    return gather_out(res.results)
